# revision 5
# baseline (speedup 1.0000x reference)
"""RBF-kernel attention on 8 TRN2 NeuronCores — v12 (paired A_bc broadcast, half-granular tail DMA).

Math (per reference): scores = exp(-gamma*SCALE*dist), dist = ||qh_s - kh_t||^2,
kept only on the STRICT upper triangle (t > s), out = scores @ vh, then @ Wo.

Factorization: scores[t,s] = exp(2c*qk[t,s] - c*qn[s] - c*kn[t]), c = gamma_h*SCALE,
with 2c folded into Wk host-side; -c*kn[t] is the exp activation's
per-partition bias; -c*qn[s] is the multiplicative A factor applied at the
attention-output copy.

v4 structure: the attention loop is scalar(exp)-bound, so ALL projection
work is streamed into its PE idle gaps instead of running as a serial
prologue:
  - prologue = khT col-chunk 0, kn chunk 0, knT tiles 0-3, qhT/A chunk 0,
    vp[0..1] only (~12us of PE) — attention chunk 0 starts immediately.
  - khT/kn/knT chunks 1-3 and vp[2..15] are emitted between t-tiles of
    attention chunk 0, each just ahead of its consumer.
  - qhT/A preamble for chunk sj+1 and the Wo matmuls (fo) of chunk sj-1
    are spread across chunk sj's t-tiles.
  - AV matmuls run one t-tile behind qk/exp.

Sharding: core c = (batch b=c//4, head-group g=c%4); each core computes 4
heads of one batch and a PARTIAL final output [S, E] (bf16) through its Wo
row slice; the host sums the 4 partials per batch. No collectives.
"""
import sys
sys.path.insert(0, '/opt/trn_rl_repo')
import math
import numpy as np
import ml_dtypes

from concourse import bass, bacc, tile, mybir, bass_utils

F32 = mybir.dt.float32
BF16 = mybir.dt.bfloat16
AF = mybir.ActivationFunctionType
ALU = mybir.AluOpType

B, S, E, H = 2, 2048, 1024, 16
D = E // H
SCALE = 1.0 / math.sqrt(D)
N_CORES = 8
HPC = H // 4
HD = HPC * D            # 256
NKT = E // 128          # 8
NST = S // 128          # 16
NSC = S // 512          # 4

_nc_cache = {}


def build_graph():
    if 'nc' in _nc_cache:
        return _nc_cache['nc']
    nc = bacc.Bacc("TRN2", target_bir_lowering=False, debug=False,
                   num_devices=N_CORES)

    qT_in = nc.dram_tensor("qTp", [128, NKT * S], BF16, kind="ExternalInput").ap()
    wq_in = nc.dram_tensor("wqp", [128, NKT * HD], BF16, kind="ExternalInput").ap()
    wk_in = nc.dram_tensor("wkp", [128, NKT * HD], BF16, kind="ExternalInput").ap()
    wv_in = nc.dram_tensor("wvp", [128, NKT * HD], BF16, kind="ExternalInput").ap()
    wo_in = nc.dram_tensor("wop", [128, 2 * E], BF16, kind="ExternalInput").ap()
    negck_in = nc.dram_tensor("negck", [2, 2], F32, kind="ExternalInput").ap()
    negcq_in = nc.dram_tensor("negcq", [2, 2], F32, kind="ExternalInput").ap()
    out_d = nc.dram_tensor("out", [S, E], BF16, kind="ExternalOutput").ap()

    id_f32 = nc.inline_tensor(np.eye(4, dtype=np.float32), name="idf32")
    hsel_np = np.zeros((128, 2), dtype=ml_dtypes.bfloat16)
    for j in range(2):
        hsel_np[64 * j:64 * j + 64, j] = 1
    hsel_c = nc.inline_tensor(hsel_np, name="hsel")
    sel64_np = np.zeros((2, 128), dtype=ml_dtypes.bfloat16)
    sel64_np[0, 0:64] = 1
    sel64_np[1, 64:128] = 1
    sel64_c = nc.inline_tensor(sel64_np, name="sel64")
    mk = (np.arange(128)[:, None] > np.arange(128)[None, :]).astype(
        ml_dtypes.bfloat16)
    mask_c = nc.inline_tensor(mk, name="mask128")

    with tile.TileContext(nc) as tc:
        with tc.tile_pool(name="persist", bufs=1) as P, \
             tc.tile_pool(name="wpool", bufs=1) as WP:
            qT = P.tile([128, NKT * S], BF16, name="qT", tag="qT")
            qhT = [P.tile([128, S], BF16, name=f"qhT{m}", tag=f"qhT{m}")
                   for m in range(2)]
            khT = [P.tile([128, S], BF16, name=f"khT{m}", tag=f"khT{m}")
                   for m in range(2)]
            vp = [P.tile([128, HD], BF16, name=f"vp{w}", tag=f"vp{w}")
                  for w in range(NST)]
            outT = [P.tile([128, S], BF16, name=f"outT{m}", tag=f"outT{m}")
                    for m in range(2)]
            kn_m = [P.tile([2, S], F32, name=f"kn{m}", tag=f"kn{m}")
                    for m in range(2)]
            knT = P.tile([128, 4 * NST], F32, name="knT", tag="knT")
            A_sb = [P.tile([2, S], BF16, name=f"Asb{m}", tag=f"Asb{m}")
                    for m in range(2)]
            A_bc = [P.tile([128, 512], BF16, name=f"Abc{m}{sj}",
                           tag=f"Abc{m}{sj}")
                    for m in range(2) for sj in range(NSC)]
            id4_t = P.tile([4, 4], F32, name="id4", tag="id4")
            hsel_t = P.tile([128, 2], BF16, name="hsel", tag="hsel")
            sel64_t = P.tile([2, 128], BF16, name="sel64", tag="sel64")
            negck_t = P.tile([2, 2], F32, name="negck", tag="negck")
            negcq_t = P.tile([2, 2], F32, name="negcq", tag="negcq")
            mask_t = P.tile([128, 128], BF16, name="mask128", tag="mask128")
            wqb = WP.tile([128, NKT * HD], BF16, name="wqb", tag="wqb")
            wkb = WP.tile([128, NKT * HD], BF16, name="wkb", tag="wkb")
            wvb = WP.tile([128, NKT * HD], BF16, name="wvb", tag="wvb")
            wob = WP.tile([128, 2 * E], BF16, name="wob", tag="wob")

            def qTs(e, lo, ln):
                # quarter-contiguous layout: [q(4)][e(8)][512]
                q, off = lo // 512, lo % 512
                col = 4096 * q + 512 * e + off
                return qT[:, col:col + ln]

            # ---- DMA: prologue-critical transfers in parallel on 3 queues ----
            nc.gpsimd.dma_start(qT[:, 0:2048], qT_in[:, 0:2048])
            nc.scalar.dma_start(qT[:, 2048:4096], qT_in[:, 2048:4096])
            nc.sync.dma_start(wkb[:], wk_in)
            nc.sync.dma_start(wqb[:], wq_in)
            nc.sync.dma_start(id4_t[:], id_f32.ap())
            nc.sync.dma_start(hsel_t[:], hsel_c.ap())
            nc.sync.dma_start(sel64_t[:], sel64_c.ap())
            nc.sync.dma_start(negck_t[:], negck_in)
            nc.sync.dma_start(negcq_t[:], negcq_in)
            nc.sync.dma_start(mask_t[:], mask_c.ap())
            nc.gpsimd.dma_start(wvb[:], wv_in)
            nc.scalar.dma_start(qT[:, 4096:6144], qT_in[:, 4096:6144])
            nc.gpsimd.dma_start(qT[:, 6144:8192], qT_in[:, 6144:8192])
            nc.scalar.dma_start(qT[:, 8192:10240], qT_in[:, 8192:10240])
            nc.gpsimd.dma_start(qT[:, 10240:12288], qT_in[:, 10240:12288])
            nc.sync.dma_start(qT[:, 12288:14336], qT_in[:, 12288:14336])
            nc.scalar.dma_start(qT[:, 14336:16384], qT_in[:, 14336:16384])
            nc.sync.dma_start(wob[:], wo_in)

            with tc.tile_pool(name="ps", bufs=6, space="PSUM") as PS, \
                 tc.tile_pool(name="otps", bufs=2, space="PSUM") as OT, \
                 tc.tile_pool(name="sq", bufs=4) as SQ, \
                 tc.tile_pool(name="ep", bufs=6) as EP:

                def emit_khT(n, m):
                    ps = PS.tile([128, 512], F32, name="ps", tag="ps")
                    for k in range(NKT):
                        nc.tensor.matmul(
                            ps[:], wkb[:, HD * k + 128 * m:
                                       HD * k + 128 * m + 128],
                            qTs(k, 512 * n, 512),
                            start=(k == 0), stop=(k == NKT - 1))
                    nc.vector.tensor_copy(
                        khT[m][:, 512 * n:512 * n + 512], ps[:])

                def emit_kn(n):
                    for m in range(2):
                        sq = SQ.tile([128, 512], BF16, name="sqk", tag="sqk")
                        nc.vector.tensor_tensor(
                            sq[:], khT[m][:, 512 * n:512 * n + 512],
                            khT[m][:, 512 * n:512 * n + 512], op=ALU.mult)
                        ps = PS.tile([128, 512], F32, name="psn", tag="ps")
                        nc.tensor.matmul(ps[0:2, :], hsel_t[:], sq[:],
                                         start=True, stop=True)
                        nc.vector.tensor_scalar(
                            kn_m[m][0:2, 512 * n:512 * n + 512], ps[0:2, :],
                            negck_t[0:2, m:m + 1], None, op0=ALU.mult)

                def emit_knT(n):
                    for ti in range(4 * n, 4 * n + 4):
                        for m in range(2):
                            ps = PS.tile([128, 512], F32, name="pst",
                                         tag="ps")
                            nc.tensor.transpose(
                                ps[0:128, 0:2],
                                kn_m[m][:, 128 * ti:128 * ti + 128],
                                id4_t[0:2, 0:2])
                            nc.vector.tensor_copy(
                                knT[:, 4 * ti + 2 * m:4 * ti + 2 * m + 2],
                                ps[0:128, 0:2])

                def emit_qhT(sj, m):
                    ps = PS.tile([128, 512], F32, name="psq", tag="ps")
                    for k in range(NKT):
                        nc.tensor.matmul(
                            ps[:], wqb[:, HD * k + 128 * m:
                                       HD * k + 128 * m + 128],
                            qTs(k, 512 * sj, 512),
                            start=(k == 0), stop=(k == NKT - 1))
                    nc.vector.tensor_copy(
                        qhT[m][:, 512 * sj:512 * sj + 512], ps[:])

                def emit_A(sj):
                    for m in range(2):
                        sq = SQ.tile([128, 512], BF16, name="sqq", tag="sqq")
                        nc.gpsimd.tensor_tensor(
                            sq[:], qhT[m][:, 512 * sj:512 * sj + 512],
                            qhT[m][:, 512 * sj:512 * sj + 512], op=ALU.mult)
                        ps = PS.tile([128, 512], F32, name="psa", tag="ps")
                        nc.tensor.matmul(ps[0:2, :], hsel_t[:], sq[:],
                                         start=True, stop=True)
                        nc.scalar.activation(
                            A_sb[m][0:2, 512 * sj:512 * sj + 512],
                            ps[0:2, :], AF.Exp,
                            scale=negcq_t[0:2, m:m + 1])
                        psb = PS.tile([128, 512], F32, name="psb",
                                      tag="ps")
                        nc.tensor.matmul(
                            psb[:], sel64_t[:],
                            A_sb[m][0:2, 512 * sj:512 * sj + 512],
                            start=True, stop=True)
                        nc.vector.tensor_copy(A_bc[m * NSC + sj][:],
                                              psb[:])

                def emit_vp(w):
                    ps = PS.tile([128, 512], F32, name="psv", tag="ps")
                    for k in range(NKT):
                        nc.tensor.matmul(
                            ps[:, 0:HD], qTs(k, 128 * w, 128),
                            wvb[:, HD * k:HD * k + HD],
                            start=(k == 0), stop=(k == NKT - 1))
                    nc.vector.tensor_copy(vp[w][:], ps[:, 0:HD])

                def emit_fo(w, tail):
                    fo = EP.tile([128, E], BF16, name="fo", tag="fo")
                    for n in range(2):
                        ps = PS.tile([128, 512], F32, name="fp", tag="ps")
                        for k in range(2):
                            nc.tensor.matmul(
                                ps[:], outT[k][:, 128 * w:128 * w + 128],
                                wob[:, E * k + 512 * n:E * k + 512 * n + 512],
                                start=(k == 0), stop=(k == 1))
                        if tail:
                            half = 512 * n
                            nc.vector.tensor_copy(
                                fo[:, half:half + 256], ps[:, 0:256])
                            nc.scalar.activation(
                                fo[:, half + 256:half + 512],
                                ps[:, 256:512], AF.Copy)
                            nc.sync.dma_start(
                                out_d[128 * w:128 * w + 128,
                                      half:half + 512],
                                fo[:, half:half + 512])
                        else:
                            nc.vector.tensor_copy(
                                fo[:, 512 * n:512 * n + 512], ps[:])
                    if not tail:
                        nc.sync.dma_start(out_d[128 * w:128 * w + 128, :],
                                          fo[:])

                def emit_outT_block(sj, wi):
                    for m in range(2):
                        for hl in range(2):
                            h = 2 * m + hl
                            base = 64 * hl
                            nc.vector.tensor_tensor(
                                outT[m][base:base + 64,
                                        512 * sj + 128 * wi:
                                        512 * sj + 128 * wi + 128],
                                ot_ps_holder[0][m][base:base + 64,
                                                   128 * wi:128 * wi + 128],
                                A_bc[m * NSC + sj][base:base + 64,
                                                   128 * wi:128 * wi + 128],
                                op=ALU.mult)

                def emit_av(ot_ps, pend, last):
                    pti, pspan, pets = pend
                    for m in range(2):
                        for hl in range(2):
                            h = 2 * m + hl
                            base = 64 * hl
                            nc.tensor.matmul(
                                ot_ps[m][base:base + 64, 0:pspan],
                                vp[pti][:, 64 * h:64 * h + 64],
                                pets[m][:, 512 * hl:512 * hl + pspan],
                                start=False, stop=last,
                                skip_group_check=True)

                ot_ps_holder = [None]

                def emit_outT(sj, per_window):
                    blocks = range(4) if per_window else [None]
                    for wi in blocks:
                        lo = 512 * sj + (128 * wi if per_window else 0)
                        ln = 128 if per_window else 512
                        alo = 128 * wi if per_window else 0
                        for m in range(2):
                            for hl in range(2):
                                h = 2 * m + hl
                                base = 64 * hl
                                nc.vector.tensor_tensor(
                                    outT[m][base:base + 64, lo:lo + ln],
                                    ot_ps_holder[0][m][base:base + 64,
                                                 alo:alo + ln],
                                    A_bc[m * NSC + sj][base:base + 64,
                                                       alo:alo + ln],
                                    op=ALU.mult)

                # ---- PE warmup: ramp the clock while input DMAs land ----
                wsc = SQ.tile([128, 256], BF16, name="wsc", tag="wsc")
                nc.vector.memset(wsc[:], 0.0)
                for _ in range(20):
                    wps = PS.tile([128, 512], F32, name="wps", tag="ps")
                    nc.tensor.matmul(wps[:, 0:256], wsc[:, 0:128], wsc[:],
                                     start=True, stop=True)

                # ---- prologue: only what attention chunk 0 t-tile 0 needs ----
                emit_khT(0, 0)
                emit_khT(0, 1)
                emit_kn(0)
                emit_knT(0)
                emit_qhT(0, 0)
                emit_qhT(0, 1)
                emit_A(0)
                emit_vp(0)
                emit_vp(1)

                # fill units per (sj, ti): list of thunks
                fills = {}
                for n in range(1, 4):
                    fills.setdefault((0, 3 * n - 2), []).append(
                        (lambda n=n: emit_khT(n, 0)))
                    fills.setdefault((0, 3 * n - 1), []).append(
                        (lambda n=n: emit_khT(n, 1)))
                    fills.setdefault((0, 3 * n), []).append(
                        (lambda n=n: (emit_kn(n), emit_knT(n))))
                for w in range(2, NST):
                    fills.setdefault((0, w - 2), []).append(
                        (lambda w=w: emit_vp(w)))
                for sj in range(1, NSC):
                    # preamble for chunk sj inside chunk sj-1 (slots must be
                    # within chunk sj-1's ti range [4(sj-1), 15])
                    base_ti = 10 if sj == 1 else 4 * sj
                    fills.setdefault((sj - 1, base_ti), []).append(
                        (lambda sj=sj: emit_qhT(sj, 0)))
                    fills.setdefault((sj - 1, base_ti + 1), []).append(
                        (lambda sj=sj: emit_qhT(sj, 1)))
                    fills.setdefault((sj - 1, base_ti + 2), []).append(
                        (lambda sj=sj: emit_A(sj)))
                for sj in range(1, NSC):
                    # fo of chunk sj-1 spread into chunk sj
                    for i, w in enumerate(range(4 * (sj - 1), 4 * sj)):
                        fills.setdefault((sj, min(4 * sj + 1 + i, NST - 1)),
                                         []).append(
                            (lambda w=w: emit_fo(w, False)))

                steps = [(sj, ti) for sj in range(NSC)
                         for ti in range(4 * sj, NST)]
                ot_of = {}
                pends_of = {}
                prev_done = None  # (sj, ot_ps, last_pend) awaiting flush
                for sj, ti in steps:
                    r = ti - 4 * sj
                    span = min(512, 128 * (r + 1))
                    diag = r < 4
                    ets = []
                    for m in range(2):
                        et2 = EP.tile([128, 1024], BF16, name="et", tag="et")
                        for hl in range(2):
                            base = 64 * hl
                            qk2 = PS.tile([128, 512], F32, name="qk",
                                          tag="ps")
                            nc.tensor.matmul(
                                qk2[:, 0:span],
                                khT[m][base:base + 64,
                                       128 * ti:128 * ti + 128],
                                qhT[m][base:base + 64,
                                       512 * sj:512 * sj + span],
                                start=True, stop=True)
                            nc.scalar.activation(
                                et2[:, 512 * hl:512 * hl + span],
                                qk2[:, 0:span], AF.Exp,
                                bias=knT[:, 4 * ti + 2 * m + hl:
                                         4 * ti + 2 * m + hl + 1])
                            if diag:
                                off = 512 * hl + 128 * r
                                nc.gpsimd.tensor_tensor(
                                    et2[:, off:off + 128],
                                    et2[:, off:off + 128],
                                    mask_t[:], op=ALU.mult)
                        ets.append(et2)

                    # deferred flush of the previous chunk (after this step's
                    # qk/exp so neither the scalar nor the PE queue drains)
                    if prev_done is not None:
                        psj, pot, plast = prev_done
                        emit_av(pot, plast, True)
                        ot_ps_holder[0] = pot
                        emit_outT(psj, per_window=False)
                        prev_done = None

                    if ti == 4 * sj:
                        # chunk start AFTER the flush: OT bufs=2 reuse needs
                        # the previous chunk's outT emitted first
                        ot_of[sj] = [OT.tile([128, 512], F32, name="ot",
                                             tag="ot") for m in range(2)]
                        for m in range(2):
                            nc.vector.memset(ot_of[sj][m][:], 0.0)
                        pends_of[sj] = []

                    pends = pends_of[sj]
                    pends.append((ti, span, ets))
                    if len(pends) > 2:
                        emit_av(ot_of[sj], pends.pop(0), False)

                    for f in fills.get((sj, ti), ()):
                        f()

                    if ti == NST - 1 and sj < NSC - 1:
                        while len(pends) > 1:
                            emit_av(ot_of[sj], pends.pop(0), False)
                        prev_done = (sj, ot_of[sj], pends.pop(0))

                # last chunk: flush + per-window outT + tail fo
                sj = NSC - 1
                while len(pends_of[sj]) > 1:
                    emit_av(ot_of[sj], pends_of[sj].pop(0), False)
                emit_av(ot_of[sj], pends_of[sj].pop(0), True)
                ot_ps_holder[0] = ot_of[sj]
                for wi in range(4):
                    emit_outT_block(sj, wi)
                    emit_fo(4 * sj + wi, True)

    nc.compile()
    _nc_cache['nc'] = nc
    return nc


def _pack(a, nblk, rows=128):
    """[nblk*rows, X] -> [rows, nblk*X] with blocks side by side."""
    x = a.shape[1]
    return np.ascontiguousarray(
        a.reshape(nblk, rows, x).transpose(1, 0, 2).reshape(rows, nblk * x))


def shard_inputs(q, Wq, Wk, Wv, Wo, gamma):
    in_maps = []
    for c in range(N_CORES):
        b, g = c // 4, c % 4
        cols = slice(HD * g, HD * (g + 1))
        gam = gamma[HPC * g:HPC * (g + 1)].astype(np.float64)
        c_h = gam * SCALE
        wk_scaled = Wk[:, cols].astype(np.float64).copy()
        for h in range(HPC):
            wk_scaled[:, 64 * h:64 * h + 64] *= 2.0 * c_h[h]
        negck = (-1.0 / (4.0 * c_h)).reshape(2, 2).T
        negcq = (-c_h).reshape(2, 2).T
        bf = ml_dtypes.bfloat16
        in_maps.append(dict(
            qTp=np.ascontiguousarray(np.ascontiguousarray(q[b].T).astype(bf).reshape(NKT, 128, NSC, 512).transpose(1, 2, 0, 3).reshape(128, NKT * S)),
            wqp=_pack(Wq[:, cols].astype(bf), NKT),
            wkp=_pack(wk_scaled.astype(np.float32).astype(bf), NKT),
            wvp=_pack(Wv[:, cols].astype(bf), NKT),
            wop=_pack(np.ascontiguousarray(Wo[cols, :]).astype(bf), 2),
            negck=np.ascontiguousarray(negck.astype(np.float32)),
            negcq=np.ascontiguousarray(negcq.astype(np.float32)),
        ))
    return in_maps


def kernel(q, Wq, Wk, Wv, Wo, gamma):
    q = np.asarray(q, dtype=np.float32)
    Wq = np.asarray(Wq, dtype=np.float32)
    Wk = np.asarray(Wk, dtype=np.float32)
    Wv = np.asarray(Wv, dtype=np.float32)
    Wo = np.asarray(Wo, dtype=np.float32)
    gamma = np.asarray(gamma, dtype=np.float32)

    nc = build_graph()
    in_maps = shard_inputs(q, Wq, Wk, Wv, Wo, gamma)
    res = bass_utils.run_bass_kernel_spmd(nc, in_maps,
                                          core_ids=list(range(N_CORES)))
    out = np.zeros((B, S, E), dtype=np.float32)
    for c in range(N_CORES):
        out[c // 4] += np.asarray(res.results[c]["out"], dtype=np.float32)
    return out


# revision 6
# speedup vs baseline: 1.1875x; 1.1875x over previous
"""RBF-kernel attention on 8 TRN2 NeuronCores — v13 (warmup matched to split-queue DMA latency).

Math (per reference): scores = exp(-gamma*SCALE*dist), dist = ||qh_s - kh_t||^2,
kept only on the STRICT upper triangle (t > s), out = scores @ vh, then @ Wo.

Factorization: scores[t,s] = exp(2c*qk[t,s] - c*qn[s] - c*kn[t]), c = gamma_h*SCALE,
with 2c folded into Wk host-side; -c*kn[t] is the exp activation's
per-partition bias; -c*qn[s] is the multiplicative A factor applied at the
attention-output copy.

v4 structure: the attention loop is scalar(exp)-bound, so ALL projection
work is streamed into its PE idle gaps instead of running as a serial
prologue:
  - prologue = khT col-chunk 0, kn chunk 0, knT tiles 0-3, qhT/A chunk 0,
    vp[0..1] only (~12us of PE) — attention chunk 0 starts immediately.
  - khT/kn/knT chunks 1-3 and vp[2..15] are emitted between t-tiles of
    attention chunk 0, each just ahead of its consumer.
  - qhT/A preamble for chunk sj+1 and the Wo matmuls (fo) of chunk sj-1
    are spread across chunk sj's t-tiles.
  - AV matmuls run one t-tile behind qk/exp.

Sharding: core c = (batch b=c//4, head-group g=c%4); each core computes 4
heads of one batch and a PARTIAL final output [S, E] (bf16) through its Wo
row slice; the host sums the 4 partials per batch. No collectives.
"""
import sys
sys.path.insert(0, '/opt/trn_rl_repo')
import math
import numpy as np
import ml_dtypes

from concourse import bass, bacc, tile, mybir, bass_utils

F32 = mybir.dt.float32
BF16 = mybir.dt.bfloat16
AF = mybir.ActivationFunctionType
ALU = mybir.AluOpType

B, S, E, H = 2, 2048, 1024, 16
D = E // H
SCALE = 1.0 / math.sqrt(D)
N_CORES = 8
HPC = H // 4
HD = HPC * D            # 256
NKT = E // 128          # 8
NST = S // 128          # 16
NSC = S // 512          # 4

_nc_cache = {}


def build_graph():
    if 'nc' in _nc_cache:
        return _nc_cache['nc']
    nc = bacc.Bacc("TRN2", target_bir_lowering=False, debug=False,
                   num_devices=N_CORES)

    qT_in = nc.dram_tensor("qTp", [128, NKT * S], BF16, kind="ExternalInput").ap()
    wq_in = nc.dram_tensor("wqp", [128, NKT * HD], BF16, kind="ExternalInput").ap()
    wk_in = nc.dram_tensor("wkp", [128, NKT * HD], BF16, kind="ExternalInput").ap()
    wv_in = nc.dram_tensor("wvp", [128, NKT * HD], BF16, kind="ExternalInput").ap()
    wo_in = nc.dram_tensor("wop", [128, 2 * E], BF16, kind="ExternalInput").ap()
    negck_in = nc.dram_tensor("negck", [2, 2], F32, kind="ExternalInput").ap()
    negcq_in = nc.dram_tensor("negcq", [2, 2], F32, kind="ExternalInput").ap()
    out_d = nc.dram_tensor("out", [S, E], BF16, kind="ExternalOutput").ap()

    id_f32 = nc.inline_tensor(np.eye(4, dtype=np.float32), name="idf32")
    hsel_np = np.zeros((128, 2), dtype=ml_dtypes.bfloat16)
    for j in range(2):
        hsel_np[64 * j:64 * j + 64, j] = 1
    hsel_c = nc.inline_tensor(hsel_np, name="hsel")
    sel64_np = np.zeros((2, 128), dtype=ml_dtypes.bfloat16)
    sel64_np[0, 0:64] = 1
    sel64_np[1, 64:128] = 1
    sel64_c = nc.inline_tensor(sel64_np, name="sel64")
    mk = (np.arange(128)[:, None] > np.arange(128)[None, :]).astype(
        ml_dtypes.bfloat16)
    mask_c = nc.inline_tensor(mk, name="mask128")

    with tile.TileContext(nc) as tc:
        with tc.tile_pool(name="persist", bufs=1) as P, \
             tc.tile_pool(name="wpool", bufs=1) as WP:
            qT = P.tile([128, NKT * S], BF16, name="qT", tag="qT")
            qhT = [P.tile([128, S], BF16, name=f"qhT{m}", tag=f"qhT{m}")
                   for m in range(2)]
            khT = [P.tile([128, S], BF16, name=f"khT{m}", tag=f"khT{m}")
                   for m in range(2)]
            vp = [P.tile([128, HD], BF16, name=f"vp{w}", tag=f"vp{w}")
                  for w in range(NST)]
            outT = [P.tile([128, S], BF16, name=f"outT{m}", tag=f"outT{m}")
                    for m in range(2)]
            kn_m = [P.tile([2, S], F32, name=f"kn{m}", tag=f"kn{m}")
                    for m in range(2)]
            knT = P.tile([128, 4 * NST], F32, name="knT", tag="knT")
            A_sb = [P.tile([2, S], BF16, name=f"Asb{m}", tag=f"Asb{m}")
                    for m in range(2)]
            A_bc = [P.tile([128, 512], BF16, name=f"Abc{m}{sj}",
                           tag=f"Abc{m}{sj}")
                    for m in range(2) for sj in range(NSC)]
            id4_t = P.tile([4, 4], F32, name="id4", tag="id4")
            hsel_t = P.tile([128, 2], BF16, name="hsel", tag="hsel")
            sel64_t = P.tile([2, 128], BF16, name="sel64", tag="sel64")
            negck_t = P.tile([2, 2], F32, name="negck", tag="negck")
            negcq_t = P.tile([2, 2], F32, name="negcq", tag="negcq")
            mask_t = P.tile([128, 128], BF16, name="mask128", tag="mask128")
            wqb = WP.tile([128, NKT * HD], BF16, name="wqb", tag="wqb")
            wkb = WP.tile([128, NKT * HD], BF16, name="wkb", tag="wkb")
            wvb = WP.tile([128, NKT * HD], BF16, name="wvb", tag="wvb")
            wob = WP.tile([128, 2 * E], BF16, name="wob", tag="wob")

            def qTs(e, lo, ln):
                # quarter-contiguous layout: [q(4)][e(8)][512]
                q, off = lo // 512, lo % 512
                col = 4096 * q + 512 * e + off
                return qT[:, col:col + ln]

            # ---- DMA: prologue-critical transfers in parallel on 3 queues ----
            nc.gpsimd.dma_start(qT[:, 0:2048], qT_in[:, 0:2048])
            nc.scalar.dma_start(qT[:, 2048:4096], qT_in[:, 2048:4096])
            nc.sync.dma_start(wkb[:], wk_in)
            nc.sync.dma_start(wqb[:], wq_in)
            nc.sync.dma_start(id4_t[:], id_f32.ap())
            nc.sync.dma_start(hsel_t[:], hsel_c.ap())
            nc.sync.dma_start(sel64_t[:], sel64_c.ap())
            nc.sync.dma_start(negck_t[:], negck_in)
            nc.sync.dma_start(negcq_t[:], negcq_in)
            nc.sync.dma_start(mask_t[:], mask_c.ap())
            nc.gpsimd.dma_start(wvb[:], wv_in)
            nc.scalar.dma_start(qT[:, 4096:6144], qT_in[:, 4096:6144])
            nc.gpsimd.dma_start(qT[:, 6144:8192], qT_in[:, 6144:8192])
            nc.scalar.dma_start(qT[:, 8192:10240], qT_in[:, 8192:10240])
            nc.gpsimd.dma_start(qT[:, 10240:12288], qT_in[:, 10240:12288])
            nc.sync.dma_start(qT[:, 12288:14336], qT_in[:, 12288:14336])
            nc.scalar.dma_start(qT[:, 14336:16384], qT_in[:, 14336:16384])
            nc.sync.dma_start(wob[:], wo_in)

            with tc.tile_pool(name="ps", bufs=6, space="PSUM") as PS, \
                 tc.tile_pool(name="otps", bufs=2, space="PSUM") as OT, \
                 tc.tile_pool(name="sq", bufs=4) as SQ, \
                 tc.tile_pool(name="ep", bufs=6) as EP:

                def emit_khT(n, m):
                    ps = PS.tile([128, 512], F32, name="ps", tag="ps")
                    for k in range(NKT):
                        nc.tensor.matmul(
                            ps[:], wkb[:, HD * k + 128 * m:
                                       HD * k + 128 * m + 128],
                            qTs(k, 512 * n, 512),
                            start=(k == 0), stop=(k == NKT - 1))
                    nc.vector.tensor_copy(
                        khT[m][:, 512 * n:512 * n + 512], ps[:])

                def emit_kn(n):
                    for m in range(2):
                        sq = SQ.tile([128, 512], BF16, name="sqk", tag="sqk")
                        nc.vector.tensor_tensor(
                            sq[:], khT[m][:, 512 * n:512 * n + 512],
                            khT[m][:, 512 * n:512 * n + 512], op=ALU.mult)
                        ps = PS.tile([128, 512], F32, name="psn", tag="ps")
                        nc.tensor.matmul(ps[0:2, :], hsel_t[:], sq[:],
                                         start=True, stop=True)
                        nc.vector.tensor_scalar(
                            kn_m[m][0:2, 512 * n:512 * n + 512], ps[0:2, :],
                            negck_t[0:2, m:m + 1], None, op0=ALU.mult)

                def emit_knT(n):
                    for ti in range(4 * n, 4 * n + 4):
                        for m in range(2):
                            ps = PS.tile([128, 512], F32, name="pst",
                                         tag="ps")
                            nc.tensor.transpose(
                                ps[0:128, 0:2],
                                kn_m[m][:, 128 * ti:128 * ti + 128],
                                id4_t[0:2, 0:2])
                            nc.vector.tensor_copy(
                                knT[:, 4 * ti + 2 * m:4 * ti + 2 * m + 2],
                                ps[0:128, 0:2])

                def emit_qhT(sj, m):
                    ps = PS.tile([128, 512], F32, name="psq", tag="ps")
                    for k in range(NKT):
                        nc.tensor.matmul(
                            ps[:], wqb[:, HD * k + 128 * m:
                                       HD * k + 128 * m + 128],
                            qTs(k, 512 * sj, 512),
                            start=(k == 0), stop=(k == NKT - 1))
                    nc.vector.tensor_copy(
                        qhT[m][:, 512 * sj:512 * sj + 512], ps[:])

                def emit_A(sj):
                    for m in range(2):
                        sq = SQ.tile([128, 512], BF16, name="sqq", tag="sqq")
                        nc.gpsimd.tensor_tensor(
                            sq[:], qhT[m][:, 512 * sj:512 * sj + 512],
                            qhT[m][:, 512 * sj:512 * sj + 512], op=ALU.mult)
                        ps = PS.tile([128, 512], F32, name="psa", tag="ps")
                        nc.tensor.matmul(ps[0:2, :], hsel_t[:], sq[:],
                                         start=True, stop=True)
                        nc.scalar.activation(
                            A_sb[m][0:2, 512 * sj:512 * sj + 512],
                            ps[0:2, :], AF.Exp,
                            scale=negcq_t[0:2, m:m + 1])
                        psb = PS.tile([128, 512], F32, name="psb",
                                      tag="ps")
                        nc.tensor.matmul(
                            psb[:], sel64_t[:],
                            A_sb[m][0:2, 512 * sj:512 * sj + 512],
                            start=True, stop=True)
                        nc.vector.tensor_copy(A_bc[m * NSC + sj][:],
                                              psb[:])

                def emit_vp(w):
                    ps = PS.tile([128, 512], F32, name="psv", tag="ps")
                    for k in range(NKT):
                        nc.tensor.matmul(
                            ps[:, 0:HD], qTs(k, 128 * w, 128),
                            wvb[:, HD * k:HD * k + HD],
                            start=(k == 0), stop=(k == NKT - 1))
                    nc.vector.tensor_copy(vp[w][:], ps[:, 0:HD])

                def emit_fo(w, tail):
                    fo = EP.tile([128, E], BF16, name="fo", tag="fo")
                    for n in range(2):
                        ps = PS.tile([128, 512], F32, name="fp", tag="ps")
                        for k in range(2):
                            nc.tensor.matmul(
                                ps[:], outT[k][:, 128 * w:128 * w + 128],
                                wob[:, E * k + 512 * n:E * k + 512 * n + 512],
                                start=(k == 0), stop=(k == 1))
                        if tail:
                            half = 512 * n
                            nc.vector.tensor_copy(
                                fo[:, half:half + 256], ps[:, 0:256])
                            nc.scalar.activation(
                                fo[:, half + 256:half + 512],
                                ps[:, 256:512], AF.Copy)
                            nc.sync.dma_start(
                                out_d[128 * w:128 * w + 128,
                                      half:half + 512],
                                fo[:, half:half + 512])
                        else:
                            nc.vector.tensor_copy(
                                fo[:, 512 * n:512 * n + 512], ps[:])
                    if not tail:
                        nc.sync.dma_start(out_d[128 * w:128 * w + 128, :],
                                          fo[:])

                def emit_outT_block(sj, wi):
                    for m in range(2):
                        for hl in range(2):
                            h = 2 * m + hl
                            base = 64 * hl
                            nc.vector.tensor_tensor(
                                outT[m][base:base + 64,
                                        512 * sj + 128 * wi:
                                        512 * sj + 128 * wi + 128],
                                ot_ps_holder[0][m][base:base + 64,
                                                   128 * wi:128 * wi + 128],
                                A_bc[m * NSC + sj][base:base + 64,
                                                   128 * wi:128 * wi + 128],
                                op=ALU.mult)

                def emit_av(ot_ps, pend, last):
                    pti, pspan, pets = pend
                    for m in range(2):
                        for hl in range(2):
                            h = 2 * m + hl
                            base = 64 * hl
                            nc.tensor.matmul(
                                ot_ps[m][base:base + 64, 0:pspan],
                                vp[pti][:, 64 * h:64 * h + 64],
                                pets[m][:, 512 * hl:512 * hl + pspan],
                                start=False, stop=last,
                                skip_group_check=True)

                ot_ps_holder = [None]

                def emit_outT(sj, per_window):
                    blocks = range(4) if per_window else [None]
                    for wi in blocks:
                        lo = 512 * sj + (128 * wi if per_window else 0)
                        ln = 128 if per_window else 512
                        alo = 128 * wi if per_window else 0
                        for m in range(2):
                            for hl in range(2):
                                h = 2 * m + hl
                                base = 64 * hl
                                nc.vector.tensor_tensor(
                                    outT[m][base:base + 64, lo:lo + ln],
                                    ot_ps_holder[0][m][base:base + 64,
                                                 alo:alo + ln],
                                    A_bc[m * NSC + sj][base:base + 64,
                                                       alo:alo + ln],
                                    op=ALU.mult)

                # ---- PE warmup: ramp the clock while input DMAs land ----
                wsc = SQ.tile([128, 256], BF16, name="wsc", tag="wsc")
                nc.vector.memset(wsc[:], 0.0)
                for _ in range(10):
                    wps = PS.tile([128, 512], F32, name="wps", tag="ps")
                    nc.tensor.matmul(wps[:, 0:256], wsc[:, 0:128], wsc[:],
                                     start=True, stop=True)

                # ---- prologue: only what attention chunk 0 t-tile 0 needs ----
                emit_khT(0, 0)
                emit_khT(0, 1)
                emit_kn(0)
                emit_knT(0)
                emit_qhT(0, 0)
                emit_qhT(0, 1)
                emit_A(0)
                emit_vp(0)
                emit_vp(1)

                # fill units per (sj, ti): list of thunks
                fills = {}
                for n in range(1, 4):
                    fills.setdefault((0, 3 * n - 2), []).append(
                        (lambda n=n: emit_khT(n, 0)))
                    fills.setdefault((0, 3 * n - 1), []).append(
                        (lambda n=n: emit_khT(n, 1)))
                    fills.setdefault((0, 3 * n), []).append(
                        (lambda n=n: (emit_kn(n), emit_knT(n))))
                for w in range(2, NST):
                    fills.setdefault((0, w - 2), []).append(
                        (lambda w=w: emit_vp(w)))
                for sj in range(1, NSC):
                    # preamble for chunk sj inside chunk sj-1 (slots must be
                    # within chunk sj-1's ti range [4(sj-1), 15])
                    base_ti = 10 if sj == 1 else 4 * sj
                    fills.setdefault((sj - 1, base_ti), []).append(
                        (lambda sj=sj: emit_qhT(sj, 0)))
                    fills.setdefault((sj - 1, base_ti + 1), []).append(
                        (lambda sj=sj: emit_qhT(sj, 1)))
                    fills.setdefault((sj - 1, base_ti + 2), []).append(
                        (lambda sj=sj: emit_A(sj)))
                for sj in range(1, NSC):
                    # fo of chunk sj-1 spread into chunk sj
                    for i, w in enumerate(range(4 * (sj - 1), 4 * sj)):
                        fills.setdefault((sj, min(4 * sj + 1 + i, NST - 1)),
                                         []).append(
                            (lambda w=w: emit_fo(w, False)))

                steps = [(sj, ti) for sj in range(NSC)
                         for ti in range(4 * sj, NST)]
                ot_of = {}
                pends_of = {}
                prev_done = None  # (sj, ot_ps, last_pend) awaiting flush
                for sj, ti in steps:
                    r = ti - 4 * sj
                    span = min(512, 128 * (r + 1))
                    diag = r < 4
                    ets = []
                    for m in range(2):
                        et2 = EP.tile([128, 1024], BF16, name="et", tag="et")
                        for hl in range(2):
                            base = 64 * hl
                            qk2 = PS.tile([128, 512], F32, name="qk",
                                          tag="ps")
                            nc.tensor.matmul(
                                qk2[:, 0:span],
                                khT[m][base:base + 64,
                                       128 * ti:128 * ti + 128],
                                qhT[m][base:base + 64,
                                       512 * sj:512 * sj + span],
                                start=True, stop=True)
                            nc.scalar.activation(
                                et2[:, 512 * hl:512 * hl + span],
                                qk2[:, 0:span], AF.Exp,
                                bias=knT[:, 4 * ti + 2 * m + hl:
                                         4 * ti + 2 * m + hl + 1])
                            if diag:
                                off = 512 * hl + 128 * r
                                nc.gpsimd.tensor_tensor(
                                    et2[:, off:off + 128],
                                    et2[:, off:off + 128],
                                    mask_t[:], op=ALU.mult)
                        ets.append(et2)

                    # deferred flush of the previous chunk (after this step's
                    # qk/exp so neither the scalar nor the PE queue drains)
                    if prev_done is not None:
                        psj, pot, plast = prev_done
                        emit_av(pot, plast, True)
                        ot_ps_holder[0] = pot
                        emit_outT(psj, per_window=False)
                        prev_done = None

                    if ti == 4 * sj:
                        # chunk start AFTER the flush: OT bufs=2 reuse needs
                        # the previous chunk's outT emitted first
                        ot_of[sj] = [OT.tile([128, 512], F32, name="ot",
                                             tag="ot") for m in range(2)]
                        for m in range(2):
                            nc.vector.memset(ot_of[sj][m][:], 0.0)
                        pends_of[sj] = []

                    pends = pends_of[sj]
                    pends.append((ti, span, ets))
                    if len(pends) > 2:
                        emit_av(ot_of[sj], pends.pop(0), False)

                    for f in fills.get((sj, ti), ()):
                        f()

                    if ti == NST - 1 and sj < NSC - 1:
                        while len(pends) > 1:
                            emit_av(ot_of[sj], pends.pop(0), False)
                        prev_done = (sj, ot_of[sj], pends.pop(0))

                # last chunk: flush + per-window outT + tail fo
                sj = NSC - 1
                while len(pends_of[sj]) > 1:
                    emit_av(ot_of[sj], pends_of[sj].pop(0), False)
                emit_av(ot_of[sj], pends_of[sj].pop(0), True)
                ot_ps_holder[0] = ot_of[sj]
                for wi in range(4):
                    emit_outT_block(sj, wi)
                    emit_fo(4 * sj + wi, True)

    nc.compile()
    _nc_cache['nc'] = nc
    return nc


def _pack(a, nblk, rows=128):
    """[nblk*rows, X] -> [rows, nblk*X] with blocks side by side."""
    x = a.shape[1]
    return np.ascontiguousarray(
        a.reshape(nblk, rows, x).transpose(1, 0, 2).reshape(rows, nblk * x))


def shard_inputs(q, Wq, Wk, Wv, Wo, gamma):
    in_maps = []
    for c in range(N_CORES):
        b, g = c // 4, c % 4
        cols = slice(HD * g, HD * (g + 1))
        gam = gamma[HPC * g:HPC * (g + 1)].astype(np.float64)
        c_h = gam * SCALE
        wk_scaled = Wk[:, cols].astype(np.float64).copy()
        for h in range(HPC):
            wk_scaled[:, 64 * h:64 * h + 64] *= 2.0 * c_h[h]
        negck = (-1.0 / (4.0 * c_h)).reshape(2, 2).T
        negcq = (-c_h).reshape(2, 2).T
        bf = ml_dtypes.bfloat16
        in_maps.append(dict(
            qTp=np.ascontiguousarray(np.ascontiguousarray(q[b].T).astype(bf).reshape(NKT, 128, NSC, 512).transpose(1, 2, 0, 3).reshape(128, NKT * S)),
            wqp=_pack(Wq[:, cols].astype(bf), NKT),
            wkp=_pack(wk_scaled.astype(np.float32).astype(bf), NKT),
            wvp=_pack(Wv[:, cols].astype(bf), NKT),
            wop=_pack(np.ascontiguousarray(Wo[cols, :]).astype(bf), 2),
            negck=np.ascontiguousarray(negck.astype(np.float32)),
            negcq=np.ascontiguousarray(negcq.astype(np.float32)),
        ))
    return in_maps


def kernel(q, Wq, Wk, Wv, Wo, gamma):
    q = np.asarray(q, dtype=np.float32)
    Wq = np.asarray(Wq, dtype=np.float32)
    Wk = np.asarray(Wk, dtype=np.float32)
    Wv = np.asarray(Wv, dtype=np.float32)
    Wo = np.asarray(Wo, dtype=np.float32)
    gamma = np.asarray(gamma, dtype=np.float32)

    nc = build_graph()
    in_maps = shard_inputs(q, Wq, Wk, Wv, Wo, gamma)
    res = bass_utils.run_bass_kernel_spmd(nc, in_maps,
                                          core_ids=list(range(N_CORES)))
    out = np.zeros((B, S, E), dtype=np.float32)
    for c in range(N_CORES):
        out[c // 4] += np.asarray(res.results[c]["out"], dtype=np.float32)
    return out
